# revision 2
# baseline (speedup 1.0000x reference)
"""Trainium2 Bass kernel v5 for nn_ActionReselector (topk_masking).

Structure (per core, 8 batches, data-parallel over B=64 across 8 cores):
  scores = city @ M_b with M_b = Wq @ k_b^T = W2 @ agent_b^T,
  W2 = Wq @ Wk^T precomputed once -> per batch setup is one transpose
  (agent_b^T) plus one matmul.

  City streams through PE twice: transpose (4 sub-blocks of 125 cities
  into one PSUM bank, ACT-evacuated once), then score matmuls
  (weight = cityT block, moving = M).  DVE does the argmax per score
  bank: 3D reduce_max for per-sub-block maxes + one FIND_INDEX8 over
  the bank (segment values are distinct fp32, no bias needed; index =
  window_pos*NA + argmax, fixed on host).

Scheduling (the part that matters):
  - DMA issue is software-pipelined: b0 (j0 quartered) + agents first,
    b1 staggered behind, then batch b+2 issued at the top of batch b.
    Per-engine SDMA queues are FIFO in desc-gen order, so issue order
    controls first-tile latency (~9us instead of ~18us).
  - Setup matmuls are phase-ordered (all aT, then all M) and emitted
    after batch 0's first transpose groups so PE never idles waiting
    on the agents DMA.
  - USE_F32R: score matmuls in f32r with the moving dim padded to 256
    (1 cyc/row fast path vs 4 for fp32); M zero-padded, score PSUM
    tiles become [125, 4*256] 2-bank super-tiles, DVE reads valid
    segments through strided views.  f32r truncates inputs (~1e-3 rel)
    which flips a small fraction of near-tie argmaxes.
"""

import sys

import numpy as np

try:
    import concourse.bacc as bacc
except ImportError:
    for _p in ("/opt/trn_rl_repo", "/root/.axon_site/_ro/trn_rl_repo"):
        if _p not in sys.path:
            sys.path.insert(0, _p)
    import concourse.bacc as bacc
import concourse.mybir as mybir
import concourse.tile as tile
from concourse import masks

B = 64
NA = 100
NC = 5000
D = 128
N_CORES = 8
B_PER_CORE = B // N_CORES

NDMA = 4
CITIES_PER_DMA = NC // NDMA       # 1250
CPP = 10                          # cities per partition per DMA tile
P_USED = CITIES_PER_DMA // CPP    # 125
NSUB = NDMA * CPP                 # 40 sub-blocks (125 cities) per batch
GRP = 4                           # sub-blocks per cityT PSUM bank group
NGRP = NSUB // GRP                # 10

F32 = mybir.dt.float32
F32R = mybir.dt.float32r
U32 = mybir.dt.uint32
AX = mybir.AxisListType

USE_F32R = True

if USE_F32R:
    GRP_SC = 4                    # sub-blocks per score super-tile
    MPAD = 256                    # moving dim incl zero pad
    SC_BUFS = 3                   # [125, 1024] 2-bank super-tiles
    CT_BUFS = 2
else:
    GRP_SC = 5
    MPAD = NA
    SC_BUFS = 3
    CT_BUFS = 3
NBANK = NSUB // GRP_SC
# f32r transposes require full 128-partition operands (ISA check rejects
# 125); pad city tiles to 128 partitions, junk rows become junk score
# rows that nothing reads.
TP = P_USED

assert P_USED * GRP * 4 <= 2048
assert MPAD * GRP_SC * 4 <= 2048 * (2 if USE_F32R else 1)


def build_nc(reps=1):
    nc = bacc.Bacc(None, target_bir_lowering=False)

    city = nc.dram_tensor("city", [B_PER_CORE, NC, D], F32, kind="ExternalInput")
    agent = nc.dram_tensor("agent", [B_PER_CORE, NA, D], F32, kind="ExternalInput")
    wq = nc.dram_tensor("wq", [D, D], F32, kind="ExternalInput")
    wk = nc.dram_tensor("wk", [D, D], F32, kind="ExternalInput")
    out = nc.dram_tensor("out", [B_PER_CORE, P_USED, NSUB], U32, kind="ExternalOutput")

    ct_dt = F32R if USE_F32R else F32
    m_dt = F32R if USE_F32R else F32

    with tile.TileContext(nc) as tc:
        with (
            tc.tile_pool(name="const", bufs=1) as constp,
            tc.tile_pool(name="weights", bufs=1) as wp,
            tc.tile_pool(name="cityin", bufs=12) as cityp,
            tc.tile_pool(name="cityT", bufs=8) as ctp,
            tc.tile_pool(name="psumCT", bufs=CT_BUFS, space="PSUM") as ctpp,
            tc.tile_pool(name="psumS", bufs=1, space="PSUM") as psp,
            tc.tile_pool(name="mmat", bufs=1) as mp,
            tc.tile_pool(name="stage", bufs=3) as stagep,
        ):
            # ---- input DMAs, in first-need order ----
            wk_sb = wp.tile([128, 128], F32)
            nc.sync.dma_start(wk_sb[:], wk[:])
            wq_sb = wp.tile([128, 128], F32)
            nc.sync.dma_start(wq_sb[:], wq[:])
            agents = wp.tile([NA, B_PER_CORE * D], F32)
            nc.sync.dma_start(agents[:], agent[:].rearrange("b a d -> a b d"))

            ident = constp.tile([128, 128], F32)
            masks.make_identity(nc, ident[:])
            if USE_F32R:
                # f32r identity for f32r transposes (affine_select cannot
                # fill f32r; ACT copy is bit-identical and f32r-rounding)
                ident_r = constp.tile([128, 128], F32R)
                nc.scalar.copy(ident_r[:], ident[:])
            else:
                ident_r = ident

            city_tiles = [[None] * NDMA for _ in range(B_PER_CORE)]

            def emit_city_dmas(b, quarters=False):
                for j in range(NDMA):
                    ctile = cityp.tile([TP, CPP * D], F32, name="ctile")
                    src = city[b, j * CITIES_PER_DMA:(j + 1) * CITIES_PER_DMA, :]
                    src2d = src.rearrange("(p n) d -> p (n d)", n=CPP)
                    if quarters and j <= 2:
                        q = CPP * D // 4
                        for qi in range(4):
                            nc.gpsimd.dma_start(
                                ctile[:P_USED, qi * q:(qi + 1) * q],
                                src2d[:, qi * q:(qi + 1) * q])
                    else:
                        nc.gpsimd.dma_start(ctile[:P_USED], src2d[:])
                    city_tiles[b][j] = ctile

            emit_city_dmas(0, quarters=True)
            emit_city_dmas(1)

            # ---- one-time: W2T = Wk @ Wq^T  (M_b = (W2T).T @ aT_b) ----
            wkT_ps = psp.tile([128, 128], F32, tag="sc", bufs=SC_BUFS)
            nc.tensor.transpose(wkT_ps[:], wk_sb[:], ident[:])
            wkT = wp.tile([128, 128], F32)
            nc.scalar.copy(wkT[:], wkT_ps[:])
            wqT_ps = psp.tile([128, 128], F32, tag="sc", bufs=SC_BUFS)
            nc.tensor.transpose(wqT_ps[:], wq_sb[:], ident[:])
            wqT = wp.tile([128, 128], F32)
            nc.scalar.copy(wqT[:], wqT_ps[:])
            w2T_ps = psp.tile([128, 128], F32, tag="sc", bufs=SC_BUFS)
            nc.tensor.matmul(w2T_ps[:], wkT[:], wqT[:], start=True, stop=True)
            w2T = wp.tile([128, 128], F32)
            nc.scalar.copy(w2T[:], w2T_ps[:])

            m_all = mp.tile([128, B_PER_CORE * MPAD], m_dt)
            if USE_F32R:
                # pad cols must be written by an f32r-rounding op (BIR
                # verifier); ACT-copy zeros from an fp32 tile.
                zpad = constp.tile([128, MPAD - NA], F32)
                nc.gpsimd.memset(zpad[:], 0.0)
                for b in range(B_PER_CORE):
                    nc.scalar.copy(
                        m_all[:, b * MPAD + NA:(b + 1) * MPAD], zpad[:])
            aTs = wp.tile([128, B_PER_CORE * NA], F32)

            # ---- steady state ----
            def emit_transpose_group(b, g):
                ctT_ps = ctpp.tile([D, GRP * TP], F32, tag="ctT")
                idarg = ident[:TP, :TP]
                for tt in range(GRP):
                    ss = g * GRP + tt
                    j, n = divmod(ss, CPP)
                    blk = city_tiles[b][j][:, n * D:(n + 1) * D]
                    nc.tensor.transpose(
                        ctT_ps[:, tt * TP:(tt + 1) * TP], blk, idarg,
                    )
                ctTs = ctp.tile([D, GRP * TP], ct_dt, name="ctTs")
                nc.scalar.copy(ctTs[:], ctT_ps[:])
                return ctTs

            def emit_setup():
                for b in range(B_PER_CORE):
                    sl = slice(b * NA, (b + 1) * NA)
                    aT_ps = psp.tile([128, NA], F32, tag="sc", bufs=SC_BUFS)
                    nc.tensor.transpose(
                        aT_ps[:], agents[:, b * D:(b + 1) * D], ident[:NA, :NA]
                    )
                    nc.scalar.copy(aTs[:, sl], aT_ps[:])
                for b in range(B_PER_CORE):
                    m_ps = psp.tile([128, NA], F32, tag="sc", bufs=SC_BUFS)
                    nc.tensor.matmul(
                        m_ps[:], w2T[:], aTs[:, b * NA:(b + 1) * NA],
                        start=True, stop=True,
                    )
                    nc.scalar.copy(m_all[:, b * MPAD:b * MPAD + NA], m_ps[:])

            def emit_batch(b, pre_groups):
                staging = stagep.tile([128, NBANK * 8], U32, tag="staging")
                stagc = stagep.tile([128, NSUB], U32, tag="stagc")
                grouped = stagep.tile([128, NSUB + 8], F32, tag="grouped")
                nc.gpsimd.memset(grouped[:P_USED, :], 0.0)

                if b + 2 < B_PER_CORE:
                    emit_city_dmas(b + 2)

                msl = slice(b * MPAD, (b + 1) * MPAD)
                ctTs_by_g = dict(pre_groups)
                sc_ps = None
                for s in range(NSUB):
                    g, t = divmod(s, GRP)
                    h, u = divmod(s, GRP_SC)
                    if t == 0 and g not in ctTs_by_g:
                        ctTs_by_g[g] = emit_transpose_group(b, g)
                    if u == 0:
                        sc_ps = psp.tile(
                            [TP, GRP_SC * MPAD], F32, tag="sc", bufs=SC_BUFS
                        )
                    lhs = ctTs_by_g[g][:, t * TP:(t + 1) * TP]
                    nc.tensor.matmul(
                        sc_ps[:, u * MPAD:(u + 1) * MPAD], lhs, m_all[:, msl],
                        start=True, stop=True, skip_group_check=True,
                    )
                    if u == GRP_SC - 1:
                        sc3d = sc_ps[:P_USED].rearrange("p (t a) -> p t a", a=MPAD)
                        vals = sc3d[:, :, 0:NA] if USE_F32R else sc_ps[:P_USED]
                        nc.vector.reduce_max(
                            grouped[:P_USED, h * GRP_SC:(h + 1) * GRP_SC],
                            sc3d[:, :, 0:NA],
                            axis=AX.X,
                        )
                        # InstMaxIndex emitted directly: the bass wrapper
                        # insists on logical-2D in_values, but FIND_INDEX8
                        # indexes the streamed element sequence, so a
                        # strided [p, t, a] view works and yields
                        # index = t*NA + a.
                        eng = nc.vector
                        eng.add_instruction(
                            mybir.InstMaxIndex(
                                name=f"I-{nc.next_id()}",
                                ins=[
                                    eng.lower_ap(
                                        grouped[:P_USED,
                                                h * GRP_SC:h * GRP_SC + 8]),
                                    eng.lower_ap(vals),
                                ],
                                outs=[
                                    eng.lower_ap(
                                        staging[:P_USED, h * 8:(h + 1) * 8])
                                ],
                            )
                        )

                nc.vector.tensor_copy(
                    stagc[:P_USED, :],
                    staging[:P_USED, :NBANK * 8].rearrange(
                        "p (h e) -> p h e", e=8)[:, :, 0:GRP_SC],
                )
                nc.sync.dma_start(out[b], stagc[:P_USED, :])

            def emit_body():
                emit_setup()
                for b in range(B_PER_CORE):
                    emit_batch(b, [])

            if reps == 1:
                emit_body()
            else:
                with tc.For_i(0, reps, 1):
                    emit_body()

    nc.finalize()
    return nc


def _unshuffle(raw: np.ndarray) -> np.ndarray:
    """[B_PER_CORE, 125, 40] u32 -> [B_PER_CORE, 5000] city-ordered."""
    offs = (NA * (np.arange(NSUB) % GRP_SC)).astype(np.uint32)
    raw = raw - offs[None, None, :]
    a = raw.reshape(B_PER_CORE, P_USED, NDMA, CPP)
    a = a.transpose(0, 2, 1, 3)
    return a.reshape(B_PER_CORE, NC)




_RUNNER = None


class _Runner:
    """Compile the bass program once; allow repeated execution.

    Mirrors concourse.bass2jax.run_bass_via_pjrt's multi-core branch, but
    keeps the jitted sharded callable so repeat calls don't recompile.
    """

    def __init__(self, reps=1):
        import jax
        from jax.experimental.shard_map import shard_map
        from jax.sharding import Mesh, NamedSharding, PartitionSpec

        import concourse.mybir as _mybir
        from concourse import bass2jax

        self.jax = jax
        self.NamedSharding = NamedSharding
        self.PartitionSpec = PartitionSpec

        bass2jax.install_neuronx_cc_hook()
        nc = build_nc(reps=reps)
        self.nc = nc
        assert nc.dbg_addr is None

        partition_name = (
            nc.partition_id_tensor.name if nc.partition_id_tensor else None
        )
        in_names, out_names, out_avals, zero_outs = [], [], [], []
        for alloc in nc.m.functions[0].allocations:
            if not isinstance(alloc, _mybir.MemoryLocationSet):
                continue
            name = alloc.memorylocations[0].name
            if alloc.kind == "ExternalInput":
                if name != partition_name:
                    in_names.append(name)
            elif alloc.kind == "ExternalOutput":
                shape = tuple(alloc.tensor_shape)
                dtype = _mybir.dt.np(alloc.dtype)
                out_names.append(name)
                out_avals.append(jax.core.ShapedArray(shape, dtype))
                zero_outs.append(np.zeros(shape, dtype))
        n_params = len(in_names)
        n_outs = len(out_avals)
        all_in_names = list(in_names) + list(out_names)
        if partition_name is not None:
            all_in_names.append(partition_name)

        self.in_names = in_names
        self.out_names = out_names
        self.out_avals = out_avals
        self.zero_outs = zero_outs
        self.n_params = n_params

        donate = tuple(range(n_params, n_params + n_outs))

        def _body(*args):
            operands = list(args)
            if partition_name is not None:
                operands.append(bass2jax.partition_id_tensor())
            outs = bass2jax._bass_exec_p.bind(
                *operands,
                out_avals=tuple(out_avals),
                in_names=tuple(all_in_names),
                out_names=tuple(out_names),
                lowering_input_output_aliases=(),
                sim_require_finite=True,
                sim_require_nnan=True,
                nc=nc,
            )
            return tuple(outs)

        devices = jax.devices()[:N_CORES]
        assert len(devices) == N_CORES
        self.mesh = Mesh(np.asarray(devices), ("core",))
        in_specs = (PartitionSpec("core"),) * (n_params + n_outs)
        out_specs = (PartitionSpec("core"),) * n_outs
        self.sharded = jax.jit(
            shard_map(
                _body,
                mesh=self.mesh,
                in_specs=in_specs,
                out_specs=out_specs,
                check_rep=False,
            ),
            donate_argnums=donate,
            keep_unused=True,
        )

    def concat_inputs(self, in_maps):
        return [
            np.concatenate(
                [np.asarray(m[name]) for m in in_maps], axis=0
            )
            for name in self.in_names
        ]

    def device_inputs(self, in_maps):
        """Pre-place concatenated inputs on the mesh (for timing loops)."""
        spec = self.NamedSharding(self.mesh, self.PartitionSpec("core"))
        return [
            self.jax.device_put(a, spec) for a in self.concat_inputs(in_maps)
        ]

    def concat_zeros(self):
        return [
            np.zeros((N_CORES * z.shape[0], *z.shape[1:]), z.dtype)
            for z in self.zero_outs
        ]

    def execute(self, placed_inputs):
        outs = self.sharded(*placed_inputs, *self.concat_zeros())
        self.jax.block_until_ready(outs)
        return outs

    def run(self, in_maps):
        out_arrs = self.execute(self.concat_inputs(in_maps))
        return [
            {
                name: np.asarray(out_arrs[i]).reshape(
                    N_CORES, *self.out_avals[i].shape
                )[c]
                for i, name in enumerate(self.out_names)
            }
            for c in range(N_CORES)
        ]



def _make_runner(reps=1):
    global _RUNNER
    if reps != 1:
        return _Runner(reps=reps)
    if _RUNNER is None:
        _RUNNER = _Runner()
    return _RUNNER


def kernel(agent_embed, city_embed, Wq, Wk):
    agent_embed = np.ascontiguousarray(np.asarray(agent_embed, dtype=np.float32))
    city_embed = np.ascontiguousarray(np.asarray(city_embed, dtype=np.float32))
    Wq = np.ascontiguousarray(np.asarray(Wq, dtype=np.float32))
    Wk = np.ascontiguousarray(np.asarray(Wk, dtype=np.float32))

    runner = _make_runner()
    in_maps = [
        {
            "city": city_embed[i * B_PER_CORE:(i + 1) * B_PER_CORE],
            "agent": agent_embed[i * B_PER_CORE:(i + 1) * B_PER_CORE],
            "wq": Wq,
            "wk": Wk,
        }
        for i in range(N_CORES)
    ]
    outs = runner.run(in_maps)
    full = np.empty((B, NC), dtype=np.int32)
    for i in range(N_CORES):
        full[i * B_PER_CORE:(i + 1) * B_PER_CORE] = _unshuffle(
            outs[i]["out"]
        ).astype(np.int32)
    return full
